# revision 28
# baseline (speedup 1.0000x reference)
"""BidafAttention Trainium2 kernel (fp8 DoubleRow + XBAR-transpose, v3).

score[b,l,r] = tanh( (lhs*w_prod) @ rhs^T + (lhs@w_l)[:,None] + (rhs@w_r)[None,:] + b )
a_lhs = softmax_R(score); a_rhs = softmax_L(score)
lhs_out = concat([lhs, a_lhs @ rhs], -1); rhs_out = concat([rhs, a_rhs^T @ lhs], -1)

Data-parallel over batch N=16 -> 2 batches per NeuronCore. All three
1024^3 GEMMs run in fp8(e4m3) DoubleRow (K=256/instr, ~170-215ns per
[128,512]-out matmul, LDWEIGHTS hidden). Scores are tanh-bounded so
softmax needs no max pass; E=exp(tanh) is fp8, att matmuls emit
UNNORMALIZED bf16 sums, host normalizes via row/col sums of e_out.

E^T (att_lhs stationary, k=r on partitions) comes from the DMA XBAR:
dma_start_transpose writes out[p,m,c] = in[c,128m+p] (image layout) on
the bf16 tanh tiles, then ACT exps the transposed tiles -> bitwise the
same values as E. No PE transposes, no PSUM->SBUF E^T copies.

Scheduling (from trace analysis):
- Inputs: big contiguous DMAs only. Strided image-chunk loads move at
  ~58GB/s and their completion order cascades through the 8-slot
  global DMA-semaphore rotation; contiguous 1MB loads are faster even
  for first-use latency.
- ACT is the scarce engine. tanh/exp/expT run as PAIRED ops (two
  128-row tiles per instruction) to amortize the ~280ns fixed ACT
  overhead; the per-partition u-bias is folded into the DVE drain
  (scalar_tensor_tensor) so tanh needs no bias read. ACT queue order
  [b0 chain, b1 jc0 tanhs, expT(0), b1 jc1 tanh/exp, expT(1)] meets
  each phase's E/E_T deadline: E(0)@~26 < attr0, E_T(0)@~36 < attl0,
  E(1)@~49 < attr1, E_T(1)@~57 < attl1.
- Phase order: score0, score1, att_rhs0, att_lhs0, att_rhs1, att_lhs1.
- DVE paces phases: S drains are [128,512] scalar_tensor_tensor ops
  (~680ns); att drains are single [128,1024] ops (~1.2us) from 2-bank
  PSUM tiles, so DVE (9.8us/phase) stays under the PE (~11.5us/phase).
- PSUM: 4x[128,512] score ring (also warmup), 2x[128,1024] att ring.
- Outputs in image layout, two row-blocks per DMA; the final block
  drains in halves (DVE+ACT concurrently) so the last DMA is small.
"""

import sys

for _p in ("/opt/trn_rl_repo",):
    if _p not in sys.path:
        sys.path.insert(0, _p)

import numpy as np
import ml_dtypes

import concourse.tile as tile
import concourse.mybir as mybir
from concourse import bacc
from concourse.bass_utils import run_bass_kernel_spmd

AF = mybir.ActivationFunctionType
ALU = mybir.AluOpType
BF16 = mybir.dt.bfloat16
F32 = mybir.dt.float32
F8 = mybir.dt.float8e4
DR = mybir.MatmulPerfMode.DoubleRow
E4 = ml_dtypes.float8_e4m3
BF = ml_dtypes.bfloat16

P = 128
SEQ = 1024  # L == R == D == 1024
NT = SEQ // P  # 8 tiles per dim
NKP = NT // 2  # 4 DoubleRow k-pairs
CH = 512  # psum chunk (free dim)
NCH = SEQ // CH  # 2
NB = 2  # batches per core
N_CORES = 8
D = 1024
SCALE = 256.0  # fold into lhsT so fp8 operands clear the subnormal range
N_WARMUP = 20  # dummy PE ops covering the ~3.5us input window (HAM gate)

_nc_cache = None


def _build_program():
    nc = bacc.Bacc("TRN2", target_bir_lowering=False, debug=False, num_devices=N_CORES)

    # inputs in SBUF-image layout: [b, p, k, cols] with row index k*128+p
    lhs_t8 = nc.declare_dram_parameter("lhs_t8", [NB, P, NT, SEQ], F8, isOutput=False)
    rhs_t8 = nc.declare_dram_parameter("rhs_t8", [NB, P, NT, SEQ], F8, isOutput=False)
    lhs_n8 = nc.declare_dram_parameter("lhs_n8", [NB, P, NT, SEQ], F8, isOutput=False)
    rhs_n8 = nc.declare_dram_parameter("rhs_n8", [NB, P, NT, SEQ], F8, isOutput=False)
    u_d = nc.declare_dram_parameter("u", [NB, P, NT], F32, isOutput=False)
    vb_d = nc.declare_dram_parameter("vb", [NB, P, SEQ], BF16, isOutput=False)
    # batch-0's jc=0 moving half, host-packed contiguous so the first
    # score sweep never waits on the (slow) strided image slice
    rhs_st_d = nc.declare_dram_parameter("rhs_st", [P, NT, CH], F8, isOutput=False)
    # outputs in image layout too; host un-permutes
    po_lhs = nc.declare_dram_parameter("po_lhs", [NB, P, NT, D], BF16, isOutput=True)
    po_rhs = nc.declare_dram_parameter("po_rhs", [NB, P, NT, D], BF16, isOutput=True)
    e_out = nc.declare_dram_parameter("e_out", [NB, P, NT, SEQ], F8, isOutput=True)

    from contextlib import ExitStack

    with tile.TileContext(nc) as tc, ExitStack() as ctx:
        const = ctx.enter_context(tc.tile_pool(name="const", bufs=1))
        pool_in = ctx.enter_context(tc.tile_pool(name="inbf", bufs=2))
        pool_e = ctx.enter_context(tc.tile_pool(name="ebf", bufs=2))
        pool_tt = ctx.enter_context(tc.tile_pool(name="ttbf", bufs=1))
        pool_T = ctx.enter_context(tc.tile_pool(name="tanh", bufs=8))
        pool_S = ctx.enter_context(tc.tile_pool(name="ssb", bufs=8))
        pool_sm = ctx.enter_context(tc.tile_pool(name="small", bufs=2))
        pool_out = ctx.enter_context(tc.tile_pool(name="osb", bufs=8))
        pool_dram = ctx.enter_context(tc.tile_pool(name="scr", bufs=1, space="DRAM"))
        psum_s = ctx.enter_context(tc.tile_pool(name="ps_s", bufs=2, space="PSUM"))
        psum_o = ctx.enter_context(tc.tile_pool(name="ps_o", bufs=3, space="PSUM"))

        # --- PE warmup: keep TensorE busy from right after the NEFF
        # preamble so the HAM clock gate opens before the first real
        # matmul (~6us in, behind the DGE cold start + 2MB loads).
        # Same byte size as the score S tiles -> shares the "ps" ring.
        wps = psum_s.tile([P, NT, P, 2], F8, tag="ps", name="warm_ps")
        wsb = const.tile([P, P], F8, name="warm_sb")
        for _ in range(N_WARMUP):
            nc.tensor.transpose(wps[:, 0, :, 0], wsb[:], wsb[:])
        # keep the warmup chain live via the DVE (ahead of its S drains);
        # the write-back also marks wsb allocated (it is never loaded)
        nc.vector.tensor_scalar_mul(wsb[:], wps[:, 0, :, 0], 1.0)

        lhsTs, rhsTs, lhs8s, rhs8s, u_sbs, vb_sbs = {}, {}, {}, {}, {}, {}
        Es, E_Ts = {}, {}
        rhs_st = const.tile([P, NT, CH], F8, name="rhs_st")
        T_pairs = {}  # (b, ip) -> [P, 2, SEQ] bf16
        S_pairs = {}  # (b, jc, ip) -> [P, 2, CH] bf16
        T_T = None

        def emit_inputs():
            for b in range(NB):
                lhsTs[b] = pool_in.tile([P, NT, SEQ], F8, tag="lhsT", name=f"lhsT{b}")
                rhsTs[b] = pool_in.tile([P, NT, SEQ], F8, tag="rhsT", name=f"rhsT{b}")
                lhs8s[b] = pool_in.tile([P, NT, SEQ], F8, tag="lhs8", name=f"lhs8{b}")
                rhs8s[b] = pool_in.tile([P, NT, SEQ], F8, tag="rhs8", name=f"rhs8{b}")
                u_sbs[b] = pool_sm.tile([P, NT], F32, tag="u", name=f"u{b}")
                vb_sbs[b] = pool_sm.tile([P, SEQ], BF16, tag="vb", name=f"vb{b}")
            sy, sc = nc.sync, nc.scalar
            # contiguous transfers only; the 0-7us HBM window belongs
            # exclusively to batch-0's score operands. Both tensors split
            # into four [P, 2-ktile, SEQ] chunks (2KB/partition, full-
            # descriptor speed, 8 parallel queues) so the kp operands of
            # the first accumulation group arrive progressively from
            # ~3.5us; everything else fires from behind a drain-dependent
            # dummy in score_mm(0).
            # first-sweep set (lhsT halves + packed rhs starter, ~1.5MB)
            # lands by ~4.6us; the full rhsT (for jc=1) rides behind it
            sy.dma_start(lhsTs[0][:, 0:4, :], lhs_t8[0, :, 0:4, :])
            sy.dma_start(lhsTs[0][:, 4:8, :], lhs_t8[0, :, 4:8, :])
            sc.dma_start(rhs_st[:], rhs_st_d[:])
            sc.dma_start(u_sbs[0][:], u_d[0])
            sc.dma_start(vb_sbs[0][:], vb_d[0])
            sc.dma_start(rhsTs[0][:], rhs_t8[0])
            sc.dma_start(u_sbs[1][:], u_d[1])
            sc.dma_start(vb_sbs[1][:], vb_d[1])

        def release_bulk(pairs, timed):
            # The scheduler reorders per-engine streams, so ring position
            # alone cannot delay bulk loads past latency-critical windows.
            # Each bulk DMA gets a real WAW dependency instead: a tiny DVE
            # write into the destination tile that READS a chosen S drain's
            # output. Releases are staggered across three drains so ~2MB
            # moves at a time and HBM never saturates against a deadline.
            for dst, src in pairs:
                nc.vector.tensor_copy(dst[:, 0, 0:4], timed)
                nc.sync.dma_start(dst[:], src)

        def emit_score_mm(b, inline_jc1_acts):
            """Matmuls + DVE drains for score(b); tanh pairs for jc=0
            inline; jc=1 tanh/exp/XBAR inline only if requested."""
            lhsT, rhsT, u_sb, vb_sb = lhsTs[b], rhsTs[b], u_sbs[b], vb_sbs[b]
            Es[b] = pool_e.tile([P, NT, SEQ], F8, tag="E", name=f"E{b}")
            E_Ts[b] = pool_e.tile([P, NT, SEQ], F8, tag="E_T", name=f"E_T{b}")
            for ip in range(NT // 2):
                T_pairs[(b, ip)] = pool_T.tile(
                    [P, 2, SEQ], BF16, tag="T", name=f"T{b}_{ip}"
                )
            for jc in range(NCH):
                for i in range(NT):
                    ip, half = i // 2, i % 2
                    S_ps = psum_s.tile([P, CH], F32, tag="ps", name=f"S{b}_{i}_{jc}")
                    for kp in range(NKP):
                        mov = (
                            rhs_st[:, 2 * kp:2 * kp + 2, :]
                            if (b == 0 and jc == 0)
                            else rhsT[:, 2 * kp:2 * kp + 2, jc * CH:(jc + 1) * CH]
                        )
                        nc.tensor.matmul(
                            S_ps[:],
                            lhsT[:, 2 * kp:2 * kp + 2, i * P:(i + 1) * P],
                            mov,
                            start=(kp == 0),
                            stop=(kp == NKP - 1),
                            perf_mode=DR,
                        )
                    if half == 0:
                        S_pairs[(b, jc, ip)] = pool_S.tile(
                            [P, 2, CH], BF16, tag="ssb", name=f"Ssb{b}_{jc}_{ip}"
                        )
                    # DVE drain folds in 256*(u[l] + v[r]); tanh needs no bias
                    nc.vector.scalar_tensor_tensor(
                        S_pairs[(b, jc, ip)][:, half, :],
                        S_ps[:],
                        u_sb[:, i:i + 1],
                        vb_sb[:, jc * CH:(jc + 1) * CH],
                        ALU.add,
                        ALU.add,
                    )
                    if jc == 0 and i == 0:
                        timed = S_pairs[(b, 0, 0)][:, 0, 0:4]
                        if b == 0:
                            release_bulk(
                                [(lhsTs[1], lhs_t8[1]), (rhsTs[1], rhs_t8[1])],
                                timed,
                            )
                        else:
                            release_bulk(
                                [(lhs8s[1], lhs_n8[1]), (rhs8s[1], rhs_n8[1])],
                                timed,
                            )
                    if b == 0 and jc == 1 and i == 0:
                        release_bulk(
                            [(lhs8s[0], lhs_n8[0]), (rhs8s[0], rhs_n8[0])],
                            S_pairs[(0, 1, 0)][:, 0, 0:4],
                        )
                    if half == 1 and (jc == 0 or inline_jc1_acts):
                        emit_tanh_pair(b, jc, ip)
                        if jc == 1 and inline_jc1_acts:
                            emit_exp_xbar_pair(b, ip)

        def emit_tanh_pair(b, jc, ip):
            # T = tanh(S_sb/256) over two row-tiles in one ACT op
            nc.scalar.activation(
                T_pairs[(b, ip)][:, 0:2, jc * CH:(jc + 1) * CH],
                S_pairs[(b, jc, ip)][:],
                AF.Tanh,
                scale=1.0 / SCALE,
            )

        def emit_exp_xbar_pair(b, ip, defer=None):
            # E = exp(T) (fp8, paired) + XBAR transposes of both tiles:
            # T^T lands in image layout [r-part, r-tile, l-cols]. With
            # defer, the XBAR triggers are handed to the att_rhs(0) DMA
            # stream so they never block output DMAs on the sync ring.
            Tp = T_pairs[(b, ip)]
            nc.scalar.activation(Es[b][:, 2 * ip:2 * ip + 2, :], Tp[:], AF.Exp)
            for h in range(2):
                i = 2 * ip + h

                def fire(i=i, Tp=Tp, h=h):
                    nc.sync.dma_start_transpose(
                        T_T[:, :, i * P:(i + 1) * P], Tp[:, h, :]
                    )

                if defer is None:
                    fire()
                else:
                    defer.append(fire)

        def emit_score_acts_jc1(b, defer=None):
            for ip in range(NT // 2):
                emit_tanh_pair(b, 1, ip)
                emit_exp_xbar_pair(b, ip, defer=defer)

        def emit_expT(b):
            # E^T = exp(T^T), paired: bitwise the same values as E
            for jp in range(NT // 2):
                nc.scalar.activation(
                    E_Ts[b][:, 2 * jp:2 * jp + 2, :],
                    T_T[:, 2 * jp:2 * jp + 2, :],
                    AF.Exp,
                )

        def emit_att_phase(stat, mov, out_d, b, tail, xbars=None):
            """One att phase: 8 output row-blocks of [128, D].
            stat: fp8 image used as DR stationary; mov: fp8 image moving.
            tail=True -> final block drains in halves with its own DMAs."""
            osb = None
            for j in range(NT):
                if j % 2 == 0:
                    osb = pool_out.tile(
                        [P, 2, SEQ], BF16, tag="osb", name=f"o{b}_{j}"
                    )
                po = psum_o.tile([P, SEQ], F32, tag="po", name=f"po{b}_{j}")
                for dc in range(NCH):
                    for kp in range(NKP):
                        nc.tensor.matmul(
                            po[:, dc * CH:(dc + 1) * CH],
                            stat[:, 2 * kp:2 * kp + 2, j * P:(j + 1) * P],
                            mov[:, 2 * kp:2 * kp + 2, dc * CH:(dc + 1) * CH],
                            start=(kp == 0),
                            stop=(kp == NKP - 1),
                            perf_mode=DR,
                        )
                if tail and j == NT - 1:
                    # halves drain on two engines AND ship on two rings,
                    # so the final DMAs run concurrently
                    nc.vector.tensor_scalar_mul(osb[:, 1, 0:CH], po[:, 0:CH], 1.0)
                    nc.sync.dma_start(
                        out_d[b, :, j:j + 1, 0:CH], osb[:, 1:2, 0:CH]
                    )
                    nc.scalar.copy(osb[:, 1, CH:SEQ], po[:, CH:SEQ])
                    nc.scalar.dma_start(
                        out_d[b, :, j:j + 1, CH:SEQ], osb[:, 1:2, CH:SEQ]
                    )
                else:
                    nc.vector.tensor_scalar_mul(osb[:, j % 2, :], po[:], 1.0)
                    if j % 2 == 1:
                        nc.sync.dma_start(out_d[b, :, j - 1:j + 1, :], osb[:])
                        if xbars:
                            xbars.pop(0)()
                            xbars.pop(0)()
                    elif tail and j == NT - 2:
                        nc.sync.dma_start(out_d[b, :, j:j + 1, :], osb[:, 0:1, :])

        # ---- emission (per-engine program order) ----
        emit_inputs()
        T_T = pool_tt.tile([P, NT, SEQ], BF16, tag="TT", name="TT")
        emit_score_mm(0, inline_jc1_acts=True)
        nc.sync.dma_start(e_out[0], Es[0][:])
        warm_dram = pool_dram.tile([P, P], F8, tag="warm", name="warm_dram")
        nc.sync.dma_start(warm_dram[:], wsb[:])
        emit_score_mm(1, inline_jc1_acts=False)
        emit_expT(0)
        xbars_b1 = []
        emit_score_acts_jc1(1, defer=xbars_b1)
        nc.scalar.dma_start(e_out[1], Es[1][:])
        emit_att_phase(Es[0], lhs8s[0], po_rhs, 0, tail=False, xbars=xbars_b1)
        # expT(1) must be emitted after att_rhs(0): the deferred b1 XBARs
        # fire there, and tile deps follow emission order
        emit_expT(1)
        emit_att_phase(E_Ts[0], rhs8s[0], po_lhs, 0, tail=False)
        emit_att_phase(Es[1], lhs8s[1], po_rhs, 1, tail=False)
        emit_att_phase(E_Ts[1], rhs8s[1], po_lhs, 1, tail=True)

    nc.compile()
    return nc


def _get_nc():
    global _nc_cache
    if _nc_cache is None:
        _nc_cache = _build_program()
    return _nc_cache


def _img(x):
    """[NB, rows, cols] -> SBUF image [NB, P, NT, cols] with row = k*128+p."""
    nb, rows, cols = x.shape
    return np.ascontiguousarray(
        x.reshape(nb, NT, P, cols).transpose(0, 2, 1, 3)
    )


def _prepare_in_maps(lhs, rhs, w, b):
    lhs = np.ascontiguousarray(lhs, dtype=np.float32)
    rhs = np.ascontiguousarray(rhs, dtype=np.float32)
    w = np.asarray(w, dtype=np.float32)
    b = np.float32(b)
    w_prod, w_l, w_r = w[:D], w[D:2 * D], w[2 * D:]

    # tiny host matvecs (exact, fp32)
    u_full = (lhs @ w_l + b) * SCALE  # (N, L), x256 domain
    v_full = rhs @ w_r                # (N, R)

    lhs_n8 = _img(lhs.astype(E4))
    rhs_n8 = _img(rhs.astype(E4))
    # d-major score operands; w_prod (x256) folds into lhs^T
    lhs_t8 = _img(
        np.ascontiguousarray((lhs * (w_prod * SCALE)).transpose(0, 2, 1)).astype(E4)
    )
    rhs_t8 = _img(np.ascontiguousarray(rhs.transpose(0, 2, 1)).astype(E4))

    in_maps = []
    for c in range(N_CORES):
        b0 = c * NB
        u_arr = np.ascontiguousarray(
            u_full[b0:b0 + NB].reshape(NB, NT, P).transpose(0, 2, 1)
        )  # (NB, 128, 8), x256 domain
        v_bf = (v_full[b0:b0 + NB] * SCALE).astype(BF)  # (NB, R), x256 domain
        vb_arr = np.ascontiguousarray(
            np.broadcast_to(v_bf[:, None, :], (NB, P, SEQ))
        )
        in_maps.append(
            {
                "lhs_t8": lhs_t8[b0:b0 + NB],
                "rhs_t8": rhs_t8[b0:b0 + NB],
                "rhs_st": np.ascontiguousarray(rhs_t8[b0, :, :, 0:CH]),
                "lhs_n8": lhs_n8[b0:b0 + NB],
                "rhs_n8": rhs_n8[b0:b0 + NB],
                "u": u_arr,
                "vb": vb_arr,
            }
        )
    return in_maps


def run_device(lhs, rhs, w, b, trace=False):
    """Returns (att_lhs, att_rhs, BassKernelResults)."""
    nc = _get_nc()
    in_maps = _prepare_in_maps(lhs, rhs, w, b)
    res = run_bass_kernel_spmd(
        nc, in_maps, core_ids=list(range(N_CORES)), trace=trace
    )
    N = lhs.shape[0]
    att_lhs = np.empty((N, SEQ, D), dtype=np.float32)
    att_rhs = np.empty((N, SEQ, D), dtype=np.float32)
    for c in range(N_CORES):
        b0 = c * NB
        # e_out image [NB, P, NT, SEQ] -> [NB, L, R]
        e = np.ascontiguousarray(
            res.results[c]["e_out"].transpose(0, 2, 1, 3)
        ).reshape(NB, SEQ, SEQ).astype(np.float32)
        rowsum = e.sum(axis=2)  # (NB, L)
        colsum = e.sum(axis=1)  # (NB, R)
        # po outputs in image layout [NB, P, NT, D] -> [NB, SEQ, D]
        pl = np.ascontiguousarray(
            res.results[c]["po_lhs"].transpose(0, 2, 1, 3)
        ).reshape(NB, SEQ, D)
        pr = np.ascontiguousarray(
            res.results[c]["po_rhs"].transpose(0, 2, 1, 3)
        ).reshape(NB, SEQ, D)
        att_lhs[b0:b0 + NB] = pl.astype(np.float32) / rowsum[:, :, None]
        att_rhs[b0:b0 + NB] = pr.astype(np.float32) / colsum[:, :, None]
    return att_lhs, att_rhs, res


def kernel(lhs, rhs, w, b):
    import os

    lhs = np.asarray(lhs, dtype=np.float32)
    rhs = np.asarray(rhs, dtype=np.float32)
    assert lhs.shape == (N_CORES * NB, SEQ, D) and rhs.shape == lhs.shape, (
        f"expected ({N_CORES * NB}, {SEQ}, {D}) inputs, got {lhs.shape}/{rhs.shape}"
    )
    had = os.environ.get("BASS_NEVER_TRACE")
    os.environ["BASS_NEVER_TRACE"] = "1"
    try:
        att_lhs, att_rhs, _ = run_device(lhs, rhs, w, b, trace=False)
    finally:
        if had is None:
            os.environ.pop("BASS_NEVER_TRACE", None)
        else:
            os.environ["BASS_NEVER_TRACE"] = had
    lhs_out = np.concatenate([lhs, att_lhs], axis=2)
    rhs_out = np.concatenate([rhs, att_rhs], axis=2)
    return lhs_out, rhs_out
